# revision 13
# baseline (speedup 1.0000x reference)
"""MixerLayerKAN Trainium2 kernel (v2).

x (B,T,C)=(32,512,512) fp32; token-mix FourierKAN(T->TD)+Linear, then
channel-mix FourierKAN(C->2C)+Linear, LN + residual around each.

Data-parallel over batch: 4 batches per NeuronCore, weights replicated,
no collectives.

Key design points (v2, rebuilt around keeping the PE p-state hot):

* Fourier features in the product basis {s, c, sc, s^2, s^3, c s^2};
  harmonic coefficients fold host-side into 6 weight matrices (channel
  KAN additionally folds its post-Linear, fp64).
* Exact range wrap to [-pi,pi]: the DVE fp32->int32 write rounds to
  nearest, so ni=round(xn/2pi), r = xn - 2pi*ni.  LN1 folds into the
  wrap/Sin AP scalars, so xn1 is never materialized.  int->float casts
  and the -2pi*nf affine run on the idle GpSimd (Pool) engine.
* ss=s^2 on ACT (Square); sc/sss/css on DVE at [128,1024] granularity,
  all bf16 (2x DVE mode).
* LN2 normalize is one DVE tensor_scalar (bf16 out, 4x mode); the
  transposes are bf16 (1 cyc/row) into bf16 PSUM tiles; the channel
  wrap chain reads PSUM directly (no ACT copies).
* Channel-mix bias via rank-1 (K=1) matmuls into the output PSUM
  accumulation; final residual add = one DVE TT (psum + x1_bf16).
* Software pipeline: PE order per batch is
    mm2(b) | mm1(b+1)[i=0,1] | T(b) | mm1(b+1)[i=2,3] | mm3(b)
  so the LN2-stat and channel-feature chains hide behind mm1 work and
  the PE never idles (TRN2 drops to 0.65/1.2GHz after any idle gap).
"""

import numpy as np
import ml_dtypes

import concourse.bass as bass
import concourse.mybir as mybir
from concourse import tile
from concourse.bass_utils import run_bass_kernel_spmd
from concourse.masks import make_identity

AF = mybir.ActivationFunctionType
OP = mybir.AluOpType
FP32 = mybir.dt.float32
BF16 = mybir.dt.bfloat16
I32 = mybir.dt.int32

B, T, C, TD, G = 32, 512, 512, 256, 3
NCORES = 8
NB = B // NCORES
P = 128
EPS = 1e-5
TWO_PI = float(2 * np.pi)
INV_2PI = float(1.0 / (2 * np.pi))
HALF_PI = float(np.pi / 2)
NF = 6
NT = T // P               # 4 t-tiles
NC_ = C // P              # 4 c-tiles
NO_TOK = TD // P          # 2 token KAN hidden tiles
FEAT_NAMES = ("s", "c", "sc", "ss", "sss", "css")


def _split_multi_waits(nc):
    """This walrus build accepts at most ONE sync-wait command per
    instruction.  Tile emits several.  Fix: before each multi-wait
    instruction, splice in same-engine NOPs carrying one wait each (a wait
    executed earlier on the same engine is semantically identical)."""
    f = nc.m.functions[0]
    per_engine = {}
    for bb in f.blocks:
        for inst in bb.instructions:
            si = getattr(inst, "sync_info", None)
            if si is not None and si.on_wait and len(si.on_wait) > 1:
                per_engine[inst.engine] = per_engine.get(inst.engine, 0) + (
                    len(si.on_wait) - 1)
    if not per_engine:
        return
    nop_pool = {}
    for eng, cnt in per_engine.items():
        nop_pool[eng] = [nc.engines[eng].nop(nofuse=True).ins for _ in range(cnt)]
    created = {id(i) for h in nop_pool.values() for i in h}
    for bb in f.blocks:
        bb.instructions[:] = [i for i in bb.instructions if id(i) not in created]
    for bb in f.blocks:
        out = []
        for inst in bb.instructions:
            si = getattr(inst, "sync_info", None)
            if si is not None and si.on_wait and len(si.on_wait) > 1:
                waits = list(si.on_wait)
                si.on_wait = [waits[-1]]
                for w in waits[:-1]:
                    nop = nop_pool[inst.engine].pop()
                    nop.sync_info = mybir.SyncInfo(on_wait=[w], on_update=[])
                    out.append(nop)
            out.append(inst)
        bb.instructions[:] = out


def _cheb_weights(coef):
    """coef (2, O, I, G) -> effective basis weights (I, 6, O) for the
    {s, c, s*c, s^2, s^3, c*s^2} basis, plus the constant term (O,).

    cos(1x)=c; cos(2x)=1-2s^2; cos(3x)=c-4c s^2
    sin(1x)=s; sin(2x)=2 s c ; sin(3x)=3s-4s^3
    """
    cosw = coef[0]
    sinw = coef[1]
    O, I, _ = cosw.shape
    w = np.zeros((I, NF, O), np.float64)
    w[:, 0, :] = (sinw[:, :, 0] + 3.0 * sinw[:, :, 2]).T      # s
    w[:, 1, :] = (cosw[:, :, 0] + cosw[:, :, 2]).T            # c
    w[:, 2, :] = (2.0 * sinw[:, :, 1]).T                      # s*c
    w[:, 3, :] = (-2.0 * cosw[:, :, 1]).T                     # s^2
    w[:, 4, :] = (-4.0 * sinw[:, :, 2]).T                     # s^3
    w[:, 5, :] = (-4.0 * cosw[:, :, 2]).T                     # c*s^2
    const = cosw[:, :, 1].sum(axis=1)
    return w, const


def _newton_rsqrt(nc, stats, var, n, tag):
    """1/sqrt(var+EPS) on DVE via bit-trick + 2 Newton iterations.
    var is a [P, n] AP; returns a [P, n] fp32 AP."""
    h = stats.tile([P, n], FP32, tag=f"{tag}h", name=f"{tag}h")
    nc.vector.tensor_scalar(out=h, in0=var, scalar1=EPS, scalar2=-0.5,
                            op0=OP.add, op1=OP.mult)
    yi = stats.tile([P, n], I32, tag=f"{tag}yi", name=f"{tag}yi")
    nc.vector.tensor_scalar(out=yi, in0=var.bitcast(I32), scalar1=1,
                            scalar2=None, op0=OP.logical_shift_right)
    nc.vector.tensor_scalar(out=yi, in0=yi, scalar1=-1,
                            scalar2=0x5F3759DF, op0=OP.mult, op1=OP.add)
    rstd = yi.bitcast(FP32)
    a = stats.tile([P, n], FP32, tag=f"{tag}a", name=f"{tag}a")
    for _ in range(2):
        nc.vector.tensor_mul(out=a, in0=rstd, in1=rstd)
        nc.vector.tensor_mul(out=a, in0=a, in1=h)
        nc.vector.scalar_tensor_tensor(out=rstd, in0=a, scalar=1.5,
                                       in1=rstd, op0=OP.add, op1=OP.mult)
    return rstd


def _build(apply_ln1, apply_ln2):
    nc = bass.Bass()
    x_in = nc.dram_tensor("x", [NB, T, C], FP32, kind="ExternalInput")
    y_out = nc.dram_tensor("y", [NB, T, C], FP32, kind="ExternalOutput")
    wtok_in = nc.dram_tensor("wtok", [NT, P, NF * TD], BF16, kind="ExternalInput")
    wchf_in = nc.dram_tensor("wchf", [NC_, P, NF * C], BF16, kind="ExternalInput")
    tlw_in = nc.dram_tensor("tlw", [NO_TOK, P, T], BF16, kind="ExternalInput")
    btok_in = nc.dram_tensor("btok", [P, NT], FP32, kind="ExternalInput")
    bch_in = nc.dram_tensor("bch", [1, C], BF16, kind="ExternalInput")
    ln_in = nc.dram_tensor("lnwb", [P, 4 * C], FP32, kind="ExternalInput")

    with tile.TileContext(nc) as tc, \
         tc.tile_pool(name="singles", bufs=1) as singles, \
         tc.tile_pool(name="xpool", bufs=2) as xpool, \
         tc.tile_pool(name="wrap", bufs=2) as wrap, \
         tc.tile_pool(name="fpool", bufs=2) as fpool, \
         tc.tile_pool(name="x1pool", bufs=2) as x1pool, \
         tc.tile_pool(name="opool", bufs=1) as opool, \
         tc.tile_pool(name="stats", bufs=2) as stats, \
         tc.tile_pool(name="psO", bufs=4, space="PSUM") as psO, \
         tc.tile_pool(name="psK", bufs=2, space="PSUM") as psK, \
         tc.tile_pool(name="psT", bufs=1, space="PSUM") as psT:

        # ---- input DMAs: X(0) and the token weights first (needed first);
        #      the rest follow ----
        Xt = {}

        def emit_x_dma(b):
            xb = xpool.tile([P, NT, C], FP32, tag="X", name=f"X{b}")
            xr = x_in[b].rearrange("(i p) c -> p i c", p=P)
            nc.sync.dma_start(out=xb[:, 0:2, :], in_=xr[:, 0:2, :])
            nc.sync.dma_start(out=xb[:, 2:4, :], in_=xr[:, 2:4, :])
            Xt[b] = [xb[:, i, :] for i in range(NT)]

        emit_x_dma(0)
        wtok_all = singles.tile([P, NT, NF, TD], BF16, tag="wtok")
        nc.sync.dma_start(
            out=wtok_all,
            in_=wtok_in.rearrange("i p (f o) -> p i f o", f=NF))
        wtok = [wtok_all[:, i] for i in range(NT)]
        tlw = []
        for j in range(NO_TOK):
            t_ = singles.tile([P, T], BF16, tag=f"tlw{j}")
            nc.sync.dma_start(out=t_, in_=tlw_in[j])
            tlw.append(t_)
        emit_x_dma(1)
        btok = singles.tile([P, NT], FP32, tag="btok")
        nc.sync.dma_start(out=btok, in_=btok_in[:, :])
        bch = singles.tile([1, C], BF16, tag="bch")
        nc.sync.dma_start(out=bch, in_=bch_in[:, :])
        wchf = []
        for m in range(NC_):
            t_ = singles.tile([P, NF, C], BF16, tag=f"wchf{m}")
            nc.sync.dma_start(out=t_, in_=wchf_in[m].rearrange("p (f o) -> p f o", f=NF))
            wchf.append(t_)
        lnwb = None
        if apply_ln1 or apply_ln2:
            lnwb = singles.tile([P, 4, C], FP32, tag="lnwb")
            nc.sync.dma_start(out=lnwb, in_=ln_in.rearrange("p (k c) -> p k c", k=4))

        identb = singles.tile([P, P], BF16, tag="identb")
        make_identity(nc, identb)
        ones1 = singles.tile([1, P], BF16, tag="ones1")
        nc.vector.memset(ones1, 1.0)
        halfpi = singles.tile([P, 1], FP32, tag="halfpi")
        nc.vector.memset(halfpi, HALF_PI)
        actwarm = singles.tile([P, 1], FP32, tag="actwarm")
        nc.scalar.activation(out=actwarm, in_=halfpi, func=AF.Sin)

        # ---- per-batch state ----
        ln1c = {}    # (r, rq, bq, negmb) columns [P, NT]
        tokf = {}    # 6 token feature tiles, each [P, 2, 1024] (pair-groups)
        ptok = {}
        ysb = {}
        pz_x1 = {}   # x1b tiles (bf16) per q
        xn2 = {}
        pT = {}      # 2 psum [P, 1024] bf16 transposed tiles
        chf = {}     # 6 channel feature tiles [P, 2, 1024]
        pout = {}

        def emit_ln1(b, split=False, pairs=(0, 1)):
            """bn stats + wrap/Sin scalar columns; split=True runs the tiny
            chain per pair so tile-0/1 features unblock early (prologue)."""
            if b in ln1c:
                mvs, st6, cols = ln1c[b]
            else:
                mvs = stats.tile([P, NT, 2], FP32, tag="mvs1", name=f"mvs1_{b}")
                st6 = stats.tile([P, 6], FP32, tag="st6", name=f"st6_{b}")
                cols = stats.tile([P, 5, NT], FP32, tag="l1cols",
                                  name=f"l1cols_{b}")
                ln1c[b] = (mvs, st6, cols)
            groups = (tuple(2 * g + k for k in range(2)) for g in pairs) \
                if split else (tuple(range(NT)),)
            for grp in groups:
                sl = slice(grp[0], grp[-1] + 1)
                for i in grp:
                    nc.vector.bn_stats(out=st6, in_=Xt[b][i])
                    nc.vector.bn_aggr(out=mvs[:, i, :], in_=st6)
                n = len(grp)
                rstd = _newton_rsqrt(nc, stats, mvs[:, sl, 1], n, f"l1g{grp[0]}")
                r_, rq, bq, negmb, nrq = (cols[:, k, sl] for k in range(5))
                nc.vector.tensor_copy(out=r_, in_=rstd)
                # negmb = -mean * rstd
                nc.vector.scalar_tensor_tensor(out=negmb, in0=mvs[:, sl, 0],
                                               scalar=-1.0, in1=rstd,
                                               op0=OP.mult, op1=OP.mult)
                nc.vector.tensor_scalar_mul(out=rq, in0=rstd, scalar1=INV_2PI)
                nc.vector.tensor_scalar_mul(out=bq, in0=negmb, scalar1=INV_2PI)
                # nrq = -2pi/rstd = -2pi*(var+eps)*rstd (rstd=(var+eps)^-1/2)
                nc.vector.tensor_scalar(out=nrq, in0=mvs[:, sl, 1], scalar1=EPS,
                                        scalar2=None, op0=OP.add)
                nc.vector.tensor_mul(out=nrq, in0=nrq, in1=rstd)
                nc.vector.tensor_scalar_mul(out=nrq, in0=nrq, scalar1=-TWO_PI)

        def emit_tok_features(b, pairs=(0, 1), dve_ab=False):
            """xn1 = (X-m)*rstd (never materialized); features via wrap+Sin.
            ni = round((X*r+negmb)/2pi) (DVE int write rounds to nearest);
            rt = X - (2pi/r)*ni (DVE stt, I32 in0 converted on read);
            s = Sin(r*rt + negmb), ab = Abs(r*rt + negmb), c = Sin(pi/2-ab)
            with the LN1 normalize folded into the ACT scale/bias.
            Emitted per pair-group g: tiles (2g, 2g+1)."""
            cols = ln1c[b][2]
            r_, rq, bq = cols[:, 0, :], cols[:, 1, :], cols[:, 2, :]
            negmb, nrq = cols[:, 3, :], cols[:, 4, :]
            if b not in tokf:
                tokf[b] = {n: fpool.tile([P, 2, 2 * C], BF16, tag=f"t{n}",
                                         name=f"t{n}_{b}")
                           for n in FEAT_NAMES}
            ft = tokf[b]
            for g in pairs:
                xn1 = {}
                if apply_ln1:
                    for i in (2 * g, 2 * g + 1):
                        xn1[i] = fpool.tile([P, C], FP32, tag=f"xn1_{i}",
                                            name=f"xn1_{b}_{i}")
                        nc.scalar.activation(out=xn1[i], in_=Xt[b][i],
                                             func=AF.Identity,
                                             bias=negmb[:, i:i + 1],
                                             scale=r_[:, i:i + 1])
                        nc.vector.tensor_mul(out=xn1[i], in0=xn1[i],
                                             in1=lnwb[:, 0, :])
                        nc.vector.tensor_add(out=xn1[i], in0=xn1[i],
                                             in1=lnwb[:, 1, :])
                ni, rt = {}, {}
                for i in (2 * g, 2 * g + 1):
                    ni[i] = wrap.tile([P, C], I32, tag=f"tni{i % 2}",
                                      name=f"tni_{b}_{i}", bufs=1)
                    if apply_ln1:
                        nc.vector.tensor_scalar_mul(out=ni[i], in0=xn1[i],
                                                    scalar1=INV_2PI)
                    else:
                        nc.vector.tensor_scalar(out=ni[i], in0=Xt[b][i],
                                                scalar1=rq[:, i:i + 1],
                                                scalar2=bq[:, i:i + 1],
                                                op0=OP.mult, op1=OP.add)
                for i in (2 * g, 2 * g + 1):
                    rt[i] = wrap.tile([P, C], BF16, tag=f"trt{i % 2}",
                                      name=f"trt_{b}_{i}", bufs=1)
                    if apply_ln1:
                        nc.vector.scalar_tensor_tensor(out=rt[i], in0=ni[i],
                                                       scalar=-TWO_PI,
                                                       in1=xn1[i],
                                                       op0=OP.mult, op1=OP.add)
                    else:
                        nc.vector.scalar_tensor_tensor(out=rt[i], in0=ni[i],
                                                       scalar=nrq[:, i:i + 1],
                                                       in1=Xt[b][i],
                                                       op0=OP.mult, op1=OP.add)
                for i in (2 * g, 2 * g + 1):
                    h = i % 2
                    sl = slice(h * C, (h + 1) * C)
                    bias = None if apply_ln1 else negmb[:, i:i + 1]
                    scale = None if apply_ln1 else r_[:, i:i + 1]
                    if dve_ab and not apply_ln1:
                        # fold scale/bias on DVE (4x mode) to offload ACT
                        rt2 = wrap.tile([P, C], BF16, tag=f"trt2_{i % 2}",
                                        name=f"trt2_{b}_{i}", bufs=1)
                        nc.vector.tensor_scalar(out=rt2, in0=rt[i],
                                                scalar1=scale, scalar2=bias,
                                                op0=OP.mult, op1=OP.add)
                        ng_ = wrap.tile([P, C], BF16, tag=f"tng{i % 2}",
                                        name=f"tng_{b}_{i}", bufs=1)
                        nc.vector.tensor_scalar_mul(out=ng_, in0=rt2,
                                                    scalar1=-1.0)
                        ab_ = wrap.tile([P, C], BF16, tag=f"tab{i % 2}",
                                        name=f"tab_{b}_{i}", bufs=1)
                        nc.vector.tensor_tensor(out=ab_, in0=rt2, in1=ng_,
                                                op=OP.max)
                        nc.scalar.activation(out=ft["s"][:, g, sl], in_=rt2,
                                             func=AF.Sin)
                        nc.scalar.activation(out=ft["c"][:, g, sl], in_=ab_,
                                             func=AF.Sin, scale=-1.0,
                                             bias=halfpi[:, :])
                    else:
                        nc.scalar.activation(out=ft["s"][:, g, sl], in_=rt[i],
                                             func=AF.Sin, bias=bias, scale=scale)
                        ab_ = wrap.tile([P, C], BF16, tag=f"tab{i % 2}",
                                        name=f"tab_{b}_{i}", bufs=1)
                        nc.scalar.activation(out=ab_, in_=rt[i], func=AF.Abs,
                                             bias=bias, scale=scale)
                        nc.scalar.activation(out=ft["c"][:, g, sl], in_=ab_,
                                             func=AF.Sin, scale=-1.0,
                                             bias=halfpi[:, :])
                if dve_ab:
                    nc.vector.tensor_mul(out=ft["ss"][:, g, :],
                                         in0=ft["s"][:, g, :],
                                         in1=ft["s"][:, g, :])
                else:
                    nc.scalar.activation(out=ft["ss"][:, g, :],
                                         in_=ft["s"][:, g, :], func=AF.Square)

        def emit_tok_muls(b, pairs=(0, 1)):
            ft = tokf[b]
            for g in pairs:
                nc.vector.tensor_mul(out=ft["sc"][:, g, :], in0=ft["s"][:, g, :],
                                     in1=ft["c"][:, g, :])
                nc.vector.tensor_mul(out=ft["sss"][:, g, :], in0=ft["ss"][:, g, :],
                                     in1=ft["s"][:, g, :])
                nc.vector.tensor_mul(out=ft["css"][:, g, :], in0=ft["ss"][:, g, :],
                                     in1=ft["c"][:, g, :])

        def emit_mm1(b, isel):
            """token KAN matmuls for t-tiles in isel."""
            if b not in ptok:
                ptok[b] = [psK.tile([P, C], FP32, tag="psk", name=f"ptok_{b}_{j}")
                           for j in range(NO_TOK)]
            ft = tokf[b]
            for i in isel:
                g, h = divmod(i, 2)
                sl = slice(h * C, (h + 1) * C)
                for f, n in enumerate(FEAT_NAMES):
                    for j in range(NO_TOK):
                        nc.tensor.matmul(ptok[b][j],
                                         wtok[i][:, f, j * P:(j + 1) * P],
                                         ft[n][:, g, sl],
                                         start=(i == 0 and f == 0),
                                         stop=(i == NT - 1 and f == NF - 1))

        def emit_ysb(b):
            ysb[b] = []
            for j in range(NO_TOK):
                y_ = fpool.tile([P, C], BF16, tag=f"ysb{j}", name=f"ysb_{b}_{j}")
                nc.scalar.copy(out=y_, in_=ptok[b][j])
                ysb[b].append(y_)

        def emit_mm2(b):
            pz_x1[b] = []
            for q in range(NT):
                if q < 2:
                    pz = psK.tile([P, C], FP32, tag="psk", name=f"pz_{b}_{q}")
                else:
                    pz = psT.tile([P, C], FP32, tag=f"pT{q - 2}",
                                  name=f"pz_{b}_{q}")
                for j in range(NO_TOK):
                    nc.tensor.matmul(pz, tlw[j][:, q * P:(q + 1) * P], ysb[b][j],
                                     start=(j == 0), stop=(j == NO_TOK - 1))
                pz_x1[b].append(pz)

        def emit_x1_ln2(b):
            """x1 = pz + btok + X (bf16 out, accum->s1); e2 via ACT Square;
            LN2 tiny chain; xn2 = x1*rstd2 + negmb2 (bf16, 4x mode)."""
            s1 = stats.tile([P, NT], FP32, tag="s1", name=f"s1_{b}")
            e2 = stats.tile([P, NT], FP32, tag="e2", name=f"e2_{b}")
            x1b = []
            for q in range(NT):
                xt_ = x1pool.tile([P, C], BF16, tag=f"x1_{q}", name=f"x1_{b}_{q}")
                nc.vector.scalar_tensor_tensor(out=xt_, in0=pz_x1[b][q],
                                               scalar=btok[:, q:q + 1],
                                               in1=Xt[b][q], op0=OP.add, op1=OP.add,
                                               accum_out=s1[:, q:q + 1])
                x1b.append(xt_)
                sq = wrap.tile([P, C], BF16, tag="sq", name=f"sq_{b}_{q}", bufs=1)
                nc.scalar.activation(out=sq, in_=xt_, func=AF.Square,
                                     accum_out=e2[:, q:q + 1])
            pz_x1[b] = x1b
            mn = stats.tile([P, NT], FP32, tag="mn", name=f"mn_{b}")
            nc.vector.tensor_scalar_mul(out=mn, in0=s1, scalar1=1.0 / C)
            vr = stats.tile([P, NT], FP32, tag="vr", name=f"vr_{b}")
            nc.vector.tensor_mul(out=vr, in0=mn, in1=mn)
            nc.vector.scalar_tensor_tensor(out=vr, in0=e2, scalar=1.0 / C,
                                           in1=vr, op0=OP.mult, op1=OP.subtract)
            rstd2 = _newton_rsqrt(nc, stats, vr, NT, "l2")
            nm2 = stats.tile([P, NT], FP32, tag="nm2", name=f"nm2_{b}")
            nc.vector.scalar_tensor_tensor(out=nm2, in0=mn, scalar=-1.0,
                                           in1=rstd2, op0=OP.mult, op1=OP.mult)
            xn2[b] = []
            for q in range(NT):
                xt_ = x1pool.tile([P, C], BF16, tag=f"xn2_{q}", name=f"xn2_{b}_{q}", bufs=1)
                nc.vector.tensor_scalar(out=xt_, in0=x1b[q],
                                        scalar1=rstd2[:, q:q + 1],
                                        scalar2=nm2[:, q:q + 1],
                                        op0=OP.mult, op1=OP.add)
                if apply_ln2:
                    nc.vector.tensor_mul(out=xt_, in0=xt_, in1=lnwb[:, 2, :])
                    nc.vector.tensor_add(out=xt_, in0=xt_, in1=lnwb[:, 3, :])
                xn2[b].append(xt_)

        def emit_T(b):
            """16 bf16 transposes into 2 bf16 psum tiles [P, 2C]; pair g
            holds m=2g (cols 0:512) and m=2g+1 (cols 512:1024), each built
            from 4 [128,128] blocks (t-tile i at columns i*128)."""
            pT[b] = [psT.tile([P, 2 * C], BF16, tag=f"pT{g}", name=f"pT_{b}_{g}",
                              padded_shape=[P, 2 * C]) for g in range(2)]
            for m in range(NC_):
                g, h = divmod(m, 2)
                for i in range(NT):
                    nc.tensor.matmul(
                        pT[b][g][:, h * C + i * P:h * C + (i + 1) * P],
                        xn2[b][i][:, m * P:(m + 1) * P], identb,
                        is_transpose=True, start=True, stop=True)

        def emit_ch_features(b, dve_heavy=False):
            """wrap + features for the transposed tiles (input in PSUM).
            Wrap ops run pair-wide [P, 2C].  dve_heavy moves abs/ss to the
            DVE for phases where ACT is the serial bottleneck."""
            ni, rt = {}, {}
            for g in range(2):
                ni[g] = wrap.tile([P, 2 * C], I32, tag=f"cni{g}",
                                  name=f"cni_{b}_{g}", bufs=1)
                nc.vector.tensor_scalar_mul(out=ni[g], in0=pT[b][g],
                                            scalar1=INV_2PI)
                rt[g] = wrap.tile([P, 2 * C], BF16, tag=f"crt{g}",
                                  name=f"crt_{b}_{g}", bufs=1)
                nc.vector.scalar_tensor_tensor(out=rt[g], in0=ni[g],
                                               scalar=-TWO_PI, in1=pT[b][g],
                                               op0=OP.mult, op1=OP.add)
            ft = {n: fpool.tile([P, 2, 2 * C], BF16, tag=f"c{n}", name=f"c{n}_{b}",
                                bufs=1)
                  for n in FEAT_NAMES}
            for g in range(2):
                if dve_heavy:
                    ng_ = wrap.tile([P, 2 * C], BF16, tag=f"cng{g}",
                                    name=f"cng_{b}_{g}", bufs=1)
                    nc.vector.tensor_scalar_mul(out=ng_, in0=rt[g], scalar1=-1.0)
                    ab_ = wrap.tile([P, 2 * C], BF16, tag=f"cab{g}",
                                    name=f"cab_{b}_{g}", bufs=1)
                    nc.vector.tensor_tensor(out=ab_, in0=rt[g], in1=ng_,
                                            op=OP.max)
                    nc.scalar.activation(out=ft["s"][:, g, :], in_=rt[g],
                                         func=AF.Sin)
                    nc.scalar.activation(out=ft["c"][:, g, :], in_=ab_,
                                         func=AF.Sin, scale=-1.0,
                                         bias=halfpi[:, :])
                    nc.vector.tensor_mul(out=ft["ss"][:, g, :],
                                         in0=ft["s"][:, g, :],
                                         in1=ft["s"][:, g, :])
                else:
                    for h in range(2):
                        m = 2 * g + h
                        sl = slice(h * C, (h + 1) * C)
                        nc.scalar.activation(out=ft["s"][:, g, sl],
                                             in_=rt[g][:, sl], func=AF.Sin)
                        ab_ = wrap.tile([P, C], BF16, tag=f"cab{m % 2}",
                                        name=f"cab_{b}_{m}", bufs=1)
                        nc.scalar.activation(out=ab_, in_=rt[g][:, sl],
                                             func=AF.Abs)
                        nc.scalar.activation(out=ft["c"][:, g, sl], in_=ab_,
                                             func=AF.Sin, scale=-1.0,
                                             bias=halfpi[:, :])
                    nc.scalar.activation(out=ft["ss"][:, g, :],
                                         in_=ft["s"][:, g, :], func=AF.Square)
                nc.vector.tensor_mul(out=ft["sc"][:, g, :], in0=ft["s"][:, g, :],
                                     in1=ft["c"][:, g, :])
                nc.vector.tensor_mul(out=ft["sss"][:, g, :], in0=ft["ss"][:, g, :],
                                     in1=ft["s"][:, g, :])
                nc.vector.tensor_mul(out=ft["css"][:, g, :], in0=ft["ss"][:, g, :],
                                     in1=ft["c"][:, g, :])
            chf[b] = ft

        def emit_mm3_head(b):
            """channel matmul m-blocks 0..2 (f/q-inner, follows feature
            production order)."""
            pout[b] = [psO.tile([P, C], FP32, tag="pso", name=f"pout_{b}_{q}")
                       for q in range(NT)]
            ft = chf[b]
            for m in range(NC_ - 1):
                g, h = divmod(m, 2)
                for f, n in enumerate(FEAT_NAMES):
                    for q in range(NT):
                        nc.tensor.matmul(pout[b][q],
                                         ft[n][:, g, h * C + q * P:h * C + (q + 1) * P],
                                         wchf[m][:, f, :],
                                         start=(m == 0 and f == 0), stop=False)

        def emit_mm3_tail(b):
            """last m-block q-outer (stop flags stagger) + rank-1 bias."""
            ft = chf[b]
            m = NC_ - 1
            g, h = divmod(m, 2)
            for q in range(NT):
                for f, n in enumerate(FEAT_NAMES):
                    nc.tensor.matmul(pout[b][q],
                                     ft[n][:, g, h * C + q * P:h * C + (q + 1) * P],
                                     wchf[m][:, f, :], start=False, stop=False)
                nc.tensor.matmul(pout[b][q], ones1, bch[0:1, :], start=False,
                                 stop=True, skip_group_check=True)

        def emit_out(b):
            for q in range(NT):
                ot = opool.tile([P, C], FP32, tag=f"out{q}", name=f"out_{b}_{q}")
                nc.vector.tensor_add(out=ot, in0=pout[b][q], in1=pz_x1[b][q])
                nc.sync.dma_start(out=y_out[b, q * P:(q + 1) * P, :], in_=ot)

        # ---- pipeline ----
        # Steady-state PE order per iteration b:
        #   mm3(b)[m0..2] | mm2(b+1) | mm3(b)[m3,q-outer] | T(b+1) |
        #   mm1(b+2)[pair0+pair1]
        # DVE order: ch(b) | ln1(b+2) | tokA-wrap(b+2) | x1/LN2(b+1) |
        #   tokA-muls | tokB(b+2) | out(b).
        # Prologue keeps DVE free of head-of-line blocks: tok(1) fully
        # emitted before x1_ln2(0) (which must wait for mm2(0)'s pz).
        emit_ln1(0, split=True, pairs=(0,))
        emit_tok_features(0, (0,), dve_ab=True)
        emit_ln1(0, split=True, pairs=(1,))
        emit_tok_features(0, (1,), dve_ab=True)
        emit_tok_muls(0, (0,))
        emit_tok_muls(0, (1,))
        emit_mm1(0, range(NT))
        emit_ysb(0)
        emit_mm2(0)
        emit_ln1(1)
        emit_tok_features(1, (0,), dve_ab=True)
        emit_tok_muls(1, (0,))
        emit_tok_features(1, (1,), dve_ab=True)
        emit_tok_muls(1, (1,))
        emit_x1_ln2(0)
        emit_T(0)
        emit_mm1(1, (0, 1))
        emit_mm1(1, (2, 3))
        emit_ysb(1)
        for b in range(NB):
            emit_ch_features(b, dve_heavy=(b == 0 or b == NB - 1))
            emit_mm3_head(b)
            if b + 1 < NB:
                emit_mm2(b + 1)
            if b + 2 < NB:
                emit_mm3_tail(b)
                emit_x_dma(b + 2)
                emit_ln1(b + 2)
                emit_tok_features(b + 2, (0,))
                emit_x1_ln2(b + 1)
                emit_T(b + 1)
                emit_tok_muls(b + 2, (0,))
                emit_tok_features(b + 2, (1,))
                emit_tok_muls(b + 2, (1,))
                emit_out(b)
                emit_mm1(b + 2, (0, 1))
                emit_mm1(b + 2, (2, 3))
                emit_ysb(b + 2)
            elif b + 1 < NB:
                # drain entry: T(b+1) before mm3_tail(b) so the channel
                # feature chain of the last batch starts ~9us earlier
                emit_x1_ln2(b + 1)
                emit_T(b + 1)
                emit_mm3_tail(b)
                emit_out(b)
            else:
                emit_mm3_tail(b)
                emit_out(b)

    _split_multi_waits(nc)
    return nc


_CACHE = {}


def _get_nc(apply_ln1, apply_ln2):
    key = (apply_ln1, apply_ln2)
    if key not in _CACHE:
        _CACHE[key] = _build(apply_ln1, apply_ln2)
    return _CACHE[key]


def prepare_in_maps(inputs):
    return _prepare(**inputs)


def _prepare(x, ln1_w, ln1_b, tok_coef, tok_kbias, tok_lw, tok_lb,
             ln2_w, ln2_b, ch_coef, ch_kbias, ch_lw, ch_lb):
    x = np.asarray(x, np.float32)
    f64 = np.float64

    wtok_eff, tok_const = _cheb_weights(np.asarray(tok_coef, f64))  # (T,6,TD)
    wch_eff, ch_const = _cheb_weights(np.asarray(ch_coef, f64))     # (C,6,2C)

    kbias_tok = np.asarray(tok_kbias, f64).reshape(-1) + tok_const
    kbias_ch = np.asarray(ch_kbias, f64).reshape(-1) + ch_const
    bias_tok = np.asarray(tok_lb, f64) + np.asarray(tok_lw, f64) @ kbias_tok
    bias_ch = np.asarray(ch_lb, f64) + np.asarray(ch_lw, f64) @ kbias_ch

    # fold the channel post-KAN linear into the KAN weights (fp64)
    wchf = np.einsum("cfo,ko->cfk", wch_eff, np.asarray(ch_lw, f64))  # (C,6,C)

    wtok_np = wtok_eff.reshape(NT, P, NF * TD).astype(ml_dtypes.bfloat16)
    wchf_np = wchf.reshape(NC_, P, NF * C).astype(ml_dtypes.bfloat16)
    tlw_np = np.ascontiguousarray(np.asarray(tok_lw, f64).T).reshape(
        NO_TOK, P, T).astype(ml_dtypes.bfloat16)
    btok_np = np.ascontiguousarray(bias_tok.reshape(NT, P).T).astype(np.float32)
    bch_np = bias_ch.reshape(1, C).astype(ml_dtypes.bfloat16)
    lnwb_np = np.broadcast_to(
        np.concatenate([np.asarray(ln1_w, f64), np.asarray(ln1_b, f64),
                        np.asarray(ln2_w, f64), np.asarray(ln2_b, f64)]).astype(
            np.float32), (P, 4 * C)).copy()

    apply_ln1 = not (np.all(np.asarray(ln1_w) == 1.0) and np.all(np.asarray(ln1_b) == 0.0))
    apply_ln2 = not (np.all(np.asarray(ln2_w) == 1.0) and np.all(np.asarray(ln2_b) == 0.0))

    shared = dict(wtok=wtok_np, wchf=wchf_np, tlw=tlw_np,
                  btok=btok_np, bch=bch_np, lnwb=lnwb_np)
    in_maps = []
    for core in range(NCORES):
        m = dict(shared)
        m["x"] = np.ascontiguousarray(x[core * NB:(core + 1) * NB])
        in_maps.append(m)
    return {"build_key": (apply_ln1, apply_ln2), "in_maps": in_maps}


def kernel(**inputs):
    prep = _prepare(**inputs)
    nc = _get_nc(*prep["build_key"])
    res = run_bass_kernel_spmd(nc, prep["in_maps"], list(range(NCORES)))
    return np.concatenate([res.results[i]["y"] for i in range(NCORES)], axis=0)


# revision 14
# speedup vs baseline: 1.1972x; 1.1972x over previous
"""MixerLayerKAN Trainium2 kernel (v2).

x (B,T,C)=(32,512,512) fp32; token-mix FourierKAN(T->TD)+Linear, then
channel-mix FourierKAN(C->2C)+Linear, LN + residual around each.

Data-parallel over batch: 4 batches per NeuronCore, weights replicated,
no collectives.

Key design points (v2, rebuilt around keeping the PE p-state hot):

* Fourier features in the product basis {s, c, sc, s^2, s^3, c s^2};
  harmonic coefficients fold host-side into 6 weight matrices (channel
  KAN additionally folds its post-Linear, fp64).
* Exact range wrap to [-pi,pi]: the DVE fp32->int32 write rounds to
  nearest, so ni=round(xn/2pi), r = xn - 2pi*ni.  LN1 folds into the
  wrap/Sin AP scalars, so xn1 is never materialized.  int->float casts
  and the -2pi*nf affine run on the idle GpSimd (Pool) engine.
* ss=s^2 on ACT (Square); sc/sss/css on DVE at [128,1024] granularity,
  all bf16 (2x DVE mode).
* LN2 normalize is one DVE tensor_scalar (bf16 out, 4x mode); the
  transposes are bf16 (1 cyc/row) into bf16 PSUM tiles; the channel
  wrap chain reads PSUM directly (no ACT copies).
* Channel-mix bias via rank-1 (K=1) matmuls into the output PSUM
  accumulation; final residual add = one DVE TT (psum + x1_bf16).
* Software pipeline: PE order per batch is
    mm2(b) | mm1(b+1)[i=0,1] | T(b) | mm1(b+1)[i=2,3] | mm3(b)
  so the LN2-stat and channel-feature chains hide behind mm1 work and
  the PE never idles (TRN2 drops to 0.65/1.2GHz after any idle gap).
"""

import numpy as np
import ml_dtypes

import concourse.bass as bass
import concourse.mybir as mybir
from concourse import tile
from concourse.bass_utils import run_bass_kernel_spmd
from concourse.masks import make_identity

AF = mybir.ActivationFunctionType
OP = mybir.AluOpType
FP32 = mybir.dt.float32
BF16 = mybir.dt.bfloat16
I32 = mybir.dt.int32

B, T, C, TD, G = 32, 512, 512, 256, 3
NCORES = 8
NB = B // NCORES
P = 128
EPS = 1e-5
TWO_PI = float(2 * np.pi)
INV_2PI = float(1.0 / (2 * np.pi))
HALF_PI = float(np.pi / 2)
NF = 6
NT = T // P               # 4 t-tiles
NC_ = C // P              # 4 c-tiles
NO_TOK = TD // P          # 2 token KAN hidden tiles
FEAT_NAMES = ("s", "c", "sc", "ss", "sss", "css")


def _split_multi_waits(nc):
    """This walrus build accepts at most ONE sync-wait command per
    instruction.  Tile emits several.  Fix: before each multi-wait
    instruction, splice in same-engine NOPs carrying one wait each (a wait
    executed earlier on the same engine is semantically identical)."""
    f = nc.m.functions[0]
    per_engine = {}
    for bb in f.blocks:
        for inst in bb.instructions:
            si = getattr(inst, "sync_info", None)
            if si is not None and si.on_wait and len(si.on_wait) > 1:
                per_engine[inst.engine] = per_engine.get(inst.engine, 0) + (
                    len(si.on_wait) - 1)
    if not per_engine:
        return
    nop_pool = {}
    for eng, cnt in per_engine.items():
        nop_pool[eng] = [nc.engines[eng].nop(nofuse=True).ins for _ in range(cnt)]
    created = {id(i) for h in nop_pool.values() for i in h}
    for bb in f.blocks:
        bb.instructions[:] = [i for i in bb.instructions if id(i) not in created]
    for bb in f.blocks:
        out = []
        for inst in bb.instructions:
            si = getattr(inst, "sync_info", None)
            if si is not None and si.on_wait and len(si.on_wait) > 1:
                waits = list(si.on_wait)
                si.on_wait = [waits[-1]]
                for w in waits[:-1]:
                    nop = nop_pool[inst.engine].pop()
                    nop.sync_info = mybir.SyncInfo(on_wait=[w], on_update=[])
                    out.append(nop)
            out.append(inst)
        bb.instructions[:] = out


def _cheb_weights(coef):
    """coef (2, O, I, G) -> effective basis weights (I, 6, O) for the
    {s, c, s*c, s^2, s^3, c*s^2} basis, plus the constant term (O,).

    cos(1x)=c; cos(2x)=1-2s^2; cos(3x)=c-4c s^2
    sin(1x)=s; sin(2x)=2 s c ; sin(3x)=3s-4s^3
    """
    cosw = coef[0]
    sinw = coef[1]
    O, I, _ = cosw.shape
    w = np.zeros((I, NF, O), np.float64)
    w[:, 0, :] = (sinw[:, :, 0] + 3.0 * sinw[:, :, 2]).T      # s
    w[:, 1, :] = (cosw[:, :, 0] + cosw[:, :, 2]).T            # c
    w[:, 2, :] = (2.0 * sinw[:, :, 1]).T                      # s*c
    w[:, 3, :] = (-2.0 * cosw[:, :, 1]).T                     # s^2
    w[:, 4, :] = (-4.0 * sinw[:, :, 2]).T                     # s^3
    w[:, 5, :] = (-4.0 * cosw[:, :, 2]).T                     # c*s^2
    const = cosw[:, :, 1].sum(axis=1)
    return w, const


def _newton_rsqrt(nc, stats, var, n, tag):
    """1/sqrt(var+EPS) on DVE via bit-trick + 2 Newton iterations.
    var is a [P, n] AP; returns a [P, n] fp32 AP."""
    h = stats.tile([P, n], FP32, tag=f"{tag}h", name=f"{tag}h")
    nc.vector.tensor_scalar(out=h, in0=var, scalar1=EPS, scalar2=-0.5,
                            op0=OP.add, op1=OP.mult)
    yi = stats.tile([P, n], I32, tag=f"{tag}yi", name=f"{tag}yi")
    nc.vector.tensor_scalar(out=yi, in0=var.bitcast(I32), scalar1=1,
                            scalar2=None, op0=OP.logical_shift_right)
    nc.vector.tensor_scalar(out=yi, in0=yi, scalar1=-1,
                            scalar2=0x5F3759DF, op0=OP.mult, op1=OP.add)
    rstd = yi.bitcast(FP32)
    a = stats.tile([P, n], FP32, tag=f"{tag}a", name=f"{tag}a")
    for _ in range(2):
        nc.vector.tensor_mul(out=a, in0=rstd, in1=rstd)
        nc.vector.tensor_mul(out=a, in0=a, in1=h)
        nc.vector.scalar_tensor_tensor(out=rstd, in0=a, scalar=1.5,
                                       in1=rstd, op0=OP.add, op1=OP.mult)
    return rstd


def _build(apply_ln1, apply_ln2):
    nc = bass.Bass()
    x_in = nc.dram_tensor("x", [NB, T, C], FP32, kind="ExternalInput")
    y_out = nc.dram_tensor("y", [NB, T, C], FP32, kind="ExternalOutput")
    wtok_in = nc.dram_tensor("wtok", [NT, P, NF * TD], BF16, kind="ExternalInput")
    wchf_in = nc.dram_tensor("wchf", [NC_, P, NF * C], BF16, kind="ExternalInput")
    tlw_in = nc.dram_tensor("tlw", [NO_TOK, P, T], BF16, kind="ExternalInput")
    btok_in = nc.dram_tensor("btok", [P, NT], FP32, kind="ExternalInput")
    bch_in = nc.dram_tensor("bch", [1, C], BF16, kind="ExternalInput")
    ln_in = nc.dram_tensor("lnwb", [P, 4 * C], FP32, kind="ExternalInput")

    with tile.TileContext(nc) as tc, \
         tc.tile_pool(name="singles", bufs=1) as singles, \
         tc.tile_pool(name="xpool", bufs=2) as xpool, \
         tc.tile_pool(name="wrap", bufs=2) as wrap, \
         tc.tile_pool(name="fpool", bufs=2) as fpool, \
         tc.tile_pool(name="x1pool", bufs=2) as x1pool, \
         tc.tile_pool(name="opool", bufs=1) as opool, \
         tc.tile_pool(name="stats", bufs=2) as stats, \
         tc.tile_pool(name="psO", bufs=4, space="PSUM") as psO, \
         tc.tile_pool(name="psK", bufs=2, space="PSUM") as psK, \
         tc.tile_pool(name="psT", bufs=1, space="PSUM") as psT:

        # ---- input DMAs: X(0) and the token weights first (needed first);
        #      the rest follow ----
        Xt = {}

        def emit_x_dma(b):
            xb = xpool.tile([P, NT, C], FP32, tag="X", name=f"X{b}")
            xr = x_in[b].rearrange("(i p) c -> p i c", p=P)
            nc.sync.dma_start(out=xb[:, 0:2, :], in_=xr[:, 0:2, :])
            nc.sync.dma_start(out=xb[:, 2:4, :], in_=xr[:, 2:4, :])
            Xt[b] = [xb[:, i, :] for i in range(NT)]

        emit_x_dma(0)
        wtok_all = singles.tile([P, NT, NF, TD], BF16, tag="wtok")
        nc.sync.dma_start(
            out=wtok_all,
            in_=wtok_in.rearrange("i p (f o) -> p i f o", f=NF))
        wtok = [wtok_all[:, i] for i in range(NT)]
        tlw = []
        for j in range(NO_TOK):
            t_ = singles.tile([P, T], BF16, tag=f"tlw{j}")
            nc.sync.dma_start(out=t_, in_=tlw_in[j])
            tlw.append(t_)
        emit_x_dma(1)
        btok = singles.tile([P, NT], FP32, tag="btok")
        nc.sync.dma_start(out=btok, in_=btok_in[:, :])
        bch = singles.tile([1, C], BF16, tag="bch")
        nc.sync.dma_start(out=bch, in_=bch_in[:, :])
        wchf = []
        for m in range(NC_):
            t_ = singles.tile([P, NF, C], BF16, tag=f"wchf{m}")
            nc.sync.dma_start(out=t_, in_=wchf_in[m].rearrange("p (f o) -> p f o", f=NF))
            wchf.append(t_)
        lnwb = None
        if apply_ln1 or apply_ln2:
            lnwb = singles.tile([P, 4, C], FP32, tag="lnwb")
            nc.sync.dma_start(out=lnwb, in_=ln_in.rearrange("p (k c) -> p k c", k=4))

        identb = singles.tile([P, P], BF16, tag="identb")
        make_identity(nc, identb)
        ones1 = singles.tile([1, P], BF16, tag="ones1")
        nc.vector.memset(ones1, 1.0)
        halfpi = singles.tile([P, 1], FP32, tag="halfpi")
        nc.vector.memset(halfpi, HALF_PI)
        actwarm = singles.tile([P, 1], FP32, tag="actwarm")
        nc.scalar.activation(out=actwarm, in_=halfpi, func=AF.Sin)

        # ---- per-batch state ----
        ln1c = {}    # (r, rq, bq, negmb) columns [P, NT]
        tokf = {}    # 6 token feature tiles, each [P, 2, 1024] (pair-groups)
        ptok = {}
        ysb = {}
        pz_x1 = {}   # x1b tiles (bf16) per q
        xn2 = {}
        pT = {}      # 2 psum [P, 1024] bf16 transposed tiles
        chf = {}     # 6 channel feature tiles [P, 2, 1024]
        pout = {}

        def emit_ln1(b, split=False, pairs=(0, 1)):
            """bn stats + wrap/Sin scalar columns; split=True runs the tiny
            chain per pair so tile-0/1 features unblock early (prologue)."""
            if b in ln1c:
                mvs, st6, cols = ln1c[b]
            else:
                mvs = stats.tile([P, NT, 2], FP32, tag="mvs1", name=f"mvs1_{b}")
                st6 = stats.tile([P, 6], FP32, tag="st6", name=f"st6_{b}")
                cols = stats.tile([P, 5, NT], FP32, tag="l1cols",
                                  name=f"l1cols_{b}")
                ln1c[b] = (mvs, st6, cols)
            groups = (tuple(2 * g + k for k in range(2)) for g in pairs) \
                if split else (tuple(range(NT)),)
            for grp in groups:
                sl = slice(grp[0], grp[-1] + 1)
                for i in grp:
                    nc.vector.bn_stats(out=st6, in_=Xt[b][i])
                    nc.vector.bn_aggr(out=mvs[:, i, :], in_=st6)
                n = len(grp)
                rstd = _newton_rsqrt(nc, stats, mvs[:, sl, 1], n, f"l1g{grp[0]}")
                r_, rq, bq, negmb, nrq = (cols[:, k, sl] for k in range(5))
                nc.vector.tensor_copy(out=r_, in_=rstd)
                # negmb = -mean * rstd
                nc.vector.scalar_tensor_tensor(out=negmb, in0=mvs[:, sl, 0],
                                               scalar=-1.0, in1=rstd,
                                               op0=OP.mult, op1=OP.mult)
                nc.vector.tensor_scalar_mul(out=rq, in0=rstd, scalar1=INV_2PI)
                nc.vector.tensor_scalar_mul(out=bq, in0=negmb, scalar1=INV_2PI)
                # nrq = -2pi/rstd = -2pi*(var+eps)*rstd (rstd=(var+eps)^-1/2)
                nc.vector.tensor_scalar(out=nrq, in0=mvs[:, sl, 1], scalar1=EPS,
                                        scalar2=None, op0=OP.add)
                nc.vector.tensor_mul(out=nrq, in0=nrq, in1=rstd)
                nc.vector.tensor_scalar_mul(out=nrq, in0=nrq, scalar1=-TWO_PI)

        def emit_tok_features(b, pairs=(0, 1)):
            """xn1 = (X-m)*rstd (never materialized); features via wrap+Sin.
            ni = round((X*r+negmb)/2pi) (DVE int write rounds to nearest);
            rt = X - (2pi/r)*ni (DVE stt, I32 in0 converted on read);
            s = Sin(r*rt + negmb), ab = Abs(r*rt + negmb), c = Sin(pi/2-ab)
            with the LN1 normalize folded into the ACT scale/bias.
            Emitted per pair-group g: tiles (2g, 2g+1)."""
            cols = ln1c[b][2]
            r_, rq, bq = cols[:, 0, :], cols[:, 1, :], cols[:, 2, :]
            negmb, nrq = cols[:, 3, :], cols[:, 4, :]
            if b not in tokf:
                tokf[b] = {n: fpool.tile([P, 2, 2 * C], BF16, tag=f"t{n}",
                                         name=f"t{n}_{b}")
                           for n in FEAT_NAMES}
            ft = tokf[b]
            for g in pairs:
                xn1 = {}
                if apply_ln1:
                    for i in (2 * g, 2 * g + 1):
                        xn1[i] = fpool.tile([P, C], FP32, tag=f"xn1_{i}",
                                            name=f"xn1_{b}_{i}")
                        nc.scalar.activation(out=xn1[i], in_=Xt[b][i],
                                             func=AF.Identity,
                                             bias=negmb[:, i:i + 1],
                                             scale=r_[:, i:i + 1])
                        nc.vector.tensor_mul(out=xn1[i], in0=xn1[i],
                                             in1=lnwb[:, 0, :])
                        nc.vector.tensor_add(out=xn1[i], in0=xn1[i],
                                             in1=lnwb[:, 1, :])
                ni, rt = {}, {}
                for i in (2 * g, 2 * g + 1):
                    ni[i] = wrap.tile([P, C], I32, tag=f"tni{i % 2}",
                                      name=f"tni_{b}_{i}", bufs=1)
                    if apply_ln1:
                        nc.vector.tensor_scalar_mul(out=ni[i], in0=xn1[i],
                                                    scalar1=INV_2PI)
                    else:
                        nc.vector.tensor_scalar(out=ni[i], in0=Xt[b][i],
                                                scalar1=rq[:, i:i + 1],
                                                scalar2=bq[:, i:i + 1],
                                                op0=OP.mult, op1=OP.add)
                for i in (2 * g, 2 * g + 1):
                    rt[i] = wrap.tile([P, C], BF16, tag=f"trt{i % 2}",
                                      name=f"trt_{b}_{i}", bufs=1)
                    if apply_ln1:
                        nc.vector.scalar_tensor_tensor(out=rt[i], in0=ni[i],
                                                       scalar=-TWO_PI,
                                                       in1=xn1[i],
                                                       op0=OP.mult, op1=OP.add)
                    else:
                        nc.vector.scalar_tensor_tensor(out=rt[i], in0=ni[i],
                                                       scalar=nrq[:, i:i + 1],
                                                       in1=Xt[b][i],
                                                       op0=OP.mult, op1=OP.add)
                for i in (2 * g, 2 * g + 1):
                    h = i % 2
                    sl = slice(h * C, (h + 1) * C)
                    bias = None if apply_ln1 else negmb[:, i:i + 1]
                    scale = None if apply_ln1 else r_[:, i:i + 1]
                    nc.scalar.activation(out=ft["s"][:, g, sl], in_=rt[i],
                                         func=AF.Sin, bias=bias, scale=scale)
                    ab_ = wrap.tile([P, C], BF16, tag=f"tab{i % 2}",
                                    name=f"tab_{b}_{i}", bufs=1)
                    nc.scalar.activation(out=ab_, in_=rt[i], func=AF.Abs,
                                         bias=bias, scale=scale)
                    nc.scalar.activation(out=ft["c"][:, g, sl], in_=ab_,
                                         func=AF.Sin, scale=-1.0,
                                         bias=halfpi[:, :])
                nc.scalar.activation(out=ft["ss"][:, g, :], in_=ft["s"][:, g, :],
                                     func=AF.Square)

        def emit_tok_muls(b, pairs=(0, 1)):
            ft = tokf[b]
            for g in pairs:
                nc.vector.tensor_mul(out=ft["sc"][:, g, :], in0=ft["s"][:, g, :],
                                     in1=ft["c"][:, g, :])
                nc.vector.tensor_mul(out=ft["sss"][:, g, :], in0=ft["ss"][:, g, :],
                                     in1=ft["s"][:, g, :])
                nc.vector.tensor_mul(out=ft["css"][:, g, :], in0=ft["ss"][:, g, :],
                                     in1=ft["c"][:, g, :])

        def emit_mm1(b, isel):
            """token KAN matmuls for t-tiles in isel."""
            if b not in ptok:
                ptok[b] = [psK.tile([P, C], FP32, tag="psk", name=f"ptok_{b}_{j}")
                           for j in range(NO_TOK)]
            ft = tokf[b]
            for i in isel:
                g, h = divmod(i, 2)
                sl = slice(h * C, (h + 1) * C)
                for f, n in enumerate(FEAT_NAMES):
                    for j in range(NO_TOK):
                        nc.tensor.matmul(ptok[b][j],
                                         wtok[i][:, f, j * P:(j + 1) * P],
                                         ft[n][:, g, sl],
                                         start=(i == 0 and f == 0),
                                         stop=(i == NT - 1 and f == NF - 1))

        def emit_ysb(b):
            ysb[b] = []
            for j in range(NO_TOK):
                y_ = fpool.tile([P, C], BF16, tag=f"ysb{j}", name=f"ysb_{b}_{j}")
                nc.scalar.copy(out=y_, in_=ptok[b][j])
                ysb[b].append(y_)

        def emit_mm2(b):
            pz_x1[b] = []
            for q in range(NT):
                if q < 2:
                    pz = psK.tile([P, C], FP32, tag="psk", name=f"pz_{b}_{q}")
                else:
                    pz = psT.tile([P, C], FP32, tag=f"pT{q - 2}",
                                  name=f"pz_{b}_{q}")
                for j in range(NO_TOK):
                    nc.tensor.matmul(pz, tlw[j][:, q * P:(q + 1) * P], ysb[b][j],
                                     start=(j == 0), stop=(j == NO_TOK - 1))
                pz_x1[b].append(pz)

        def emit_x1_ln2(b):
            """x1 = pz + btok + X (bf16 out, accum->s1); e2 via ACT Square;
            LN2 tiny chain; xn2 = x1*rstd2 + negmb2 (bf16, 4x mode)."""
            s1 = stats.tile([P, NT], FP32, tag="s1", name=f"s1_{b}")
            e2 = stats.tile([P, NT], FP32, tag="e2", name=f"e2_{b}")
            x1b = []
            for q in range(NT):
                xt_ = x1pool.tile([P, C], BF16, tag=f"x1_{q}", name=f"x1_{b}_{q}")
                nc.vector.scalar_tensor_tensor(out=xt_, in0=pz_x1[b][q],
                                               scalar=btok[:, q:q + 1],
                                               in1=Xt[b][q], op0=OP.add, op1=OP.add,
                                               accum_out=s1[:, q:q + 1])
                x1b.append(xt_)
                sq = wrap.tile([P, C], BF16, tag="sq", name=f"sq_{b}_{q}", bufs=1)
                nc.scalar.activation(out=sq, in_=xt_, func=AF.Square,
                                     accum_out=e2[:, q:q + 1])
            pz_x1[b] = x1b
            mn = stats.tile([P, NT], FP32, tag="mn", name=f"mn_{b}")
            nc.vector.tensor_scalar_mul(out=mn, in0=s1, scalar1=1.0 / C)
            vr = stats.tile([P, NT], FP32, tag="vr", name=f"vr_{b}")
            nc.vector.tensor_mul(out=vr, in0=mn, in1=mn)
            nc.vector.scalar_tensor_tensor(out=vr, in0=e2, scalar=1.0 / C,
                                           in1=vr, op0=OP.mult, op1=OP.subtract)
            rstd2 = _newton_rsqrt(nc, stats, vr, NT, "l2")
            nm2 = stats.tile([P, NT], FP32, tag="nm2", name=f"nm2_{b}")
            nc.vector.scalar_tensor_tensor(out=nm2, in0=mn, scalar=-1.0,
                                           in1=rstd2, op0=OP.mult, op1=OP.mult)
            xn2[b] = []
            for q in range(NT):
                xt_ = x1pool.tile([P, C], BF16, tag=f"xn2_{q}", name=f"xn2_{b}_{q}", bufs=1)
                nc.vector.tensor_scalar(out=xt_, in0=x1b[q],
                                        scalar1=rstd2[:, q:q + 1],
                                        scalar2=nm2[:, q:q + 1],
                                        op0=OP.mult, op1=OP.add)
                if apply_ln2:
                    nc.vector.tensor_mul(out=xt_, in0=xt_, in1=lnwb[:, 2, :])
                    nc.vector.tensor_add(out=xt_, in0=xt_, in1=lnwb[:, 3, :])
                xn2[b].append(xt_)

        def emit_T(b):
            """16 bf16 transposes into 2 bf16 psum tiles [P, 2C]; pair g
            holds m=2g (cols 0:512) and m=2g+1 (cols 512:1024), each built
            from 4 [128,128] blocks (t-tile i at columns i*128)."""
            pT[b] = [psT.tile([P, 2 * C], BF16, tag=f"pT{g}", name=f"pT_{b}_{g}",
                              padded_shape=[P, 2 * C]) for g in range(2)]
            for m in range(NC_):
                g, h = divmod(m, 2)
                for i in range(NT):
                    nc.tensor.matmul(
                        pT[b][g][:, h * C + i * P:h * C + (i + 1) * P],
                        xn2[b][i][:, m * P:(m + 1) * P], identb,
                        is_transpose=True, start=True, stop=True)

        def emit_ch_features(b):
            """wrap + features for the transposed tiles (input in PSUM)."""
            ni, rt = {}, {}
            for m in range(NC_):
                g, h = divmod(m, 2)
                z = pT[b][g][:, h * C:(h + 1) * C]
                ni[m] = wrap.tile([P, C], I32, tag=f"cni{m % 2}", name=f"cni_{b}_{m}", bufs=1)
                nc.vector.tensor_scalar_mul(out=ni[m], in0=z, scalar1=INV_2PI)
            for m in range(NC_):
                g, h = divmod(m, 2)
                z = pT[b][g][:, h * C:(h + 1) * C]
                rt[m] = wrap.tile([P, C], BF16, tag=f"crt{m % 2}", name=f"crt_{b}_{m}", bufs=1)
                nc.vector.scalar_tensor_tensor(out=rt[m], in0=ni[m], scalar=-TWO_PI,
                                               in1=z, op0=OP.mult, op1=OP.add)
            ft = {n: fpool.tile([P, 2, 2 * C], BF16, tag=f"c{n}", name=f"c{n}_{b}",
                                bufs=1)
                  for n in FEAT_NAMES}
            ab = {}
            for m in range(NC_):
                g, h = divmod(m, 2)
                sl = slice(h * C, (h + 1) * C)
                nc.scalar.activation(out=ft["s"][:, g, sl], in_=rt[m], func=AF.Sin)
                ab[m] = wrap.tile([P, C], BF16, tag=f"cab{m % 2}", name=f"cab_{b}_{m}", bufs=1)
                nc.scalar.activation(out=ab[m], in_=rt[m], func=AF.Abs)
                nc.scalar.activation(out=ft["c"][:, g, sl], in_=ab[m], func=AF.Sin,
                                     scale=-1.0, bias=halfpi[:, :])
            for g in range(2):
                nc.vector.tensor_mul(out=ft["sc"][:, g, :], in0=ft["s"][:, g, :],
                                     in1=ft["c"][:, g, :])
                nc.scalar.activation(out=ft["ss"][:, g, :], in_=ft["s"][:, g, :],
                                     func=AF.Square)
                nc.vector.tensor_mul(out=ft["sss"][:, g, :], in0=ft["ss"][:, g, :],
                                     in1=ft["s"][:, g, :])
                nc.vector.tensor_mul(out=ft["css"][:, g, :], in0=ft["ss"][:, g, :],
                                     in1=ft["c"][:, g, :])
            chf[b] = ft

        def emit_mm3_head(b):
            """channel matmul m-blocks 0..2 (f/q-inner, follows feature
            production order)."""
            pout[b] = [psO.tile([P, C], FP32, tag="pso", name=f"pout_{b}_{q}")
                       for q in range(NT)]
            ft = chf[b]
            for m in range(NC_ - 1):
                g, h = divmod(m, 2)
                for f, n in enumerate(FEAT_NAMES):
                    for q in range(NT):
                        nc.tensor.matmul(pout[b][q],
                                         ft[n][:, g, h * C + q * P:h * C + (q + 1) * P],
                                         wchf[m][:, f, :],
                                         start=(m == 0 and f == 0), stop=False)

        def emit_mm3_tail(b):
            """last m-block q-outer (stop flags stagger) + rank-1 bias."""
            ft = chf[b]
            m = NC_ - 1
            g, h = divmod(m, 2)
            for q in range(NT):
                for f, n in enumerate(FEAT_NAMES):
                    nc.tensor.matmul(pout[b][q],
                                     ft[n][:, g, h * C + q * P:h * C + (q + 1) * P],
                                     wchf[m][:, f, :], start=False, stop=False)
                nc.tensor.matmul(pout[b][q], ones1, bch[0:1, :], start=False,
                                 stop=True, skip_group_check=True)

        def emit_out(b):
            for q in range(NT):
                ot = opool.tile([P, C], FP32, tag=f"out{q}", name=f"out_{b}_{q}")
                nc.vector.tensor_add(out=ot, in0=pout[b][q], in1=pz_x1[b][q])
                nc.sync.dma_start(out=y_out[b, q * P:(q + 1) * P, :], in_=ot)

        # ---- pipeline ----
        # Steady-state PE order per iteration b:
        #   mm3(b)[m0..2] | mm2(b+1) | mm3(b)[m3,q-outer] | T(b+1) |
        #   mm1(b+2)[pair0+pair1]
        # DVE order: ch(b) | ln1(b+2) | tokA-wrap(b+2) | x1/LN2(b+1) |
        #   tokA-muls | tokB(b+2) | out(b).
        # Prologue keeps DVE free of head-of-line blocks: tok(1) fully
        # emitted before x1_ln2(0) (which must wait for mm2(0)'s pz).
        emit_ln1(0, split=True, pairs=(0,))
        emit_tok_features(0, (0,))
        emit_ln1(0, split=True, pairs=(1,))
        emit_tok_features(0, (1,))
        emit_tok_muls(0, (0,))
        emit_tok_muls(0, (1,))
        emit_mm1(0, range(NT))
        emit_ysb(0)
        emit_mm2(0)
        emit_ln1(1)
        emit_tok_features(1, (0,))
        emit_tok_muls(1, (0,))
        emit_tok_features(1, (1,))
        emit_tok_muls(1, (1,))
        emit_x1_ln2(0)
        emit_T(0)
        emit_mm1(1, (0, 1))
        emit_mm1(1, (2, 3))
        emit_ysb(1)
        for b in range(NB):
            emit_ch_features(b)
            emit_mm3_head(b)
            if b + 1 < NB:
                emit_mm2(b + 1)
            if b + 2 < NB:
                emit_mm3_tail(b)
                emit_x_dma(b + 2)
                emit_ln1(b + 2)
                emit_tok_features(b + 2, (0,))
                emit_x1_ln2(b + 1)
                emit_T(b + 1)
                emit_tok_muls(b + 2, (0,))
                emit_tok_features(b + 2, (1,))
                emit_tok_muls(b + 2, (1,))
                emit_out(b)
                emit_mm1(b + 2, (0, 1))
                emit_mm1(b + 2, (2, 3))
                emit_ysb(b + 2)
            elif b + 1 < NB:
                # drain entry: T(b+1) before mm3_tail(b) so the channel
                # feature chain of the last batch starts ~9us earlier
                emit_x1_ln2(b + 1)
                emit_T(b + 1)
                emit_mm3_tail(b)
                emit_out(b)
            else:
                emit_mm3_tail(b)
                emit_out(b)

    _split_multi_waits(nc)
    return nc


_CACHE = {}


def _get_nc(apply_ln1, apply_ln2):
    key = (apply_ln1, apply_ln2)
    if key not in _CACHE:
        _CACHE[key] = _build(apply_ln1, apply_ln2)
    return _CACHE[key]


def prepare_in_maps(inputs):
    return _prepare(**inputs)


def _prepare(x, ln1_w, ln1_b, tok_coef, tok_kbias, tok_lw, tok_lb,
             ln2_w, ln2_b, ch_coef, ch_kbias, ch_lw, ch_lb):
    x = np.asarray(x, np.float32)
    f64 = np.float64

    wtok_eff, tok_const = _cheb_weights(np.asarray(tok_coef, f64))  # (T,6,TD)
    wch_eff, ch_const = _cheb_weights(np.asarray(ch_coef, f64))     # (C,6,2C)

    kbias_tok = np.asarray(tok_kbias, f64).reshape(-1) + tok_const
    kbias_ch = np.asarray(ch_kbias, f64).reshape(-1) + ch_const
    bias_tok = np.asarray(tok_lb, f64) + np.asarray(tok_lw, f64) @ kbias_tok
    bias_ch = np.asarray(ch_lb, f64) + np.asarray(ch_lw, f64) @ kbias_ch

    # fold the channel post-KAN linear into the KAN weights (fp64)
    wchf = np.einsum("cfo,ko->cfk", wch_eff, np.asarray(ch_lw, f64))  # (C,6,C)

    wtok_np = wtok_eff.reshape(NT, P, NF * TD).astype(ml_dtypes.bfloat16)
    wchf_np = wchf.reshape(NC_, P, NF * C).astype(ml_dtypes.bfloat16)
    tlw_np = np.ascontiguousarray(np.asarray(tok_lw, f64).T).reshape(
        NO_TOK, P, T).astype(ml_dtypes.bfloat16)
    btok_np = np.ascontiguousarray(bias_tok.reshape(NT, P).T).astype(np.float32)
    bch_np = bias_ch.reshape(1, C).astype(ml_dtypes.bfloat16)
    lnwb_np = np.broadcast_to(
        np.concatenate([np.asarray(ln1_w, f64), np.asarray(ln1_b, f64),
                        np.asarray(ln2_w, f64), np.asarray(ln2_b, f64)]).astype(
            np.float32), (P, 4 * C)).copy()

    apply_ln1 = not (np.all(np.asarray(ln1_w) == 1.0) and np.all(np.asarray(ln1_b) == 0.0))
    apply_ln2 = not (np.all(np.asarray(ln2_w) == 1.0) and np.all(np.asarray(ln2_b) == 0.0))

    shared = dict(wtok=wtok_np, wchf=wchf_np, tlw=tlw_np,
                  btok=btok_np, bch=bch_np, lnwb=lnwb_np)
    in_maps = []
    for core in range(NCORES):
        m = dict(shared)
        m["x"] = np.ascontiguousarray(x[core * NB:(core + 1) * NB])
        in_maps.append(m)
    return {"build_key": (apply_ln1, apply_ln2), "in_maps": in_maps}


def kernel(**inputs):
    prep = _prepare(**inputs)
    nc = _get_nc(*prep["build_key"])
    res = run_bass_kernel_spmd(nc, prep["in_maps"], list(range(NCORES)))
    return np.concatenate([res.results[i]["y"] for i in range(NCORES)], axis=0)
